# revision 1
# baseline (speedup 1.0000x reference)
"""Trainium2 Bass kernel for nn_DistinctionLoss (GFTT corners BCE + relu-cosine mean).

v2: batch-sharded 2 images/core across 8 cores.
 - fp8 DoubleRow raw gram (host-side e4m3 cast + d-major slab layout);
   normalization folded in post-relu via N=1 PE matvecs with r = rsqrt(diag).
 - GFTT restructured: (dx^2+dy^2, dx^2-dy^2) pushed through the linear gaussian
   convs, deleting the tr/A add/sub stages.
 - Elementwise spread across Act/DVE/Pool; bf16 everywhere DVE gets 2x mode.
"""
import os
import numpy as np
import ml_dtypes

import concourse.bacc as bacc
import concourse.mybir as mybir
from concourse.tile import TileContext
from concourse.bass_utils import run_bass_kernel_spmd

F32 = mybir.dt.float32
BF16 = mybir.dt.bfloat16
FP8 = mybir.dt.float8e4
AF = mybir.ActivationFunctionType
ALU = mybir.AluOpType
AX = mybir.AxisListType
DR = mybir.MatmulPerfMode.DoubleRow

H = W = 384
NIMG = 2
NDESC = 2048
DDIM = 256
NUM = 200
NEG = -1e30
BW = 136  # packed band window width

_bf = lambda a: np.ascontiguousarray(a.astype(ml_dtypes.bfloat16))


def _band(k, mode, n=384):
    pad = len(k) // 2
    idx = np.arange(n + 2 * pad) - pad
    if mode == "edge":
        src = np.clip(idx, 0, n - 1)
    else:  # reflect
        src = np.abs(idx)
        src = np.where(src >= n, 2 * (n - 1) - src, src)
    M = np.zeros((n, n), np.float32)
    for i, kv in enumerate(k):
        M[src[np.arange(n) + i], np.arange(n)] += kv
    return M


def _gauss7():
    xs = np.arange(7, dtype=np.float32) - 3.0
    g = np.exp(-0.5 * xs ** 2)
    return (g / g.sum()).astype(np.float32)


def _wins(M, nchunk):
    wins = []
    for k in range(nchunk):
        rows = M[k * 128:(k + 1) * 128]
        nz = np.nonzero(np.any(rows != 0, axis=0))[0]
        wins.append((int(nz[0]), int(nz[-1]) + 1) if len(nz) else None)
    return wins


def _pack(M, nchunk, wins):
    """Pack band matrix rows into [nchunk*128, BW] windows."""
    P = np.zeros((nchunk * 128, BW), np.float32)
    for k in range(nchunk):
        if wins[k] is None:
            continue
        c0, c1 = wins[k]
        P[k * 128:(k + 1) * 128, 0:c1 - c0] = M[k * 128:(k + 1) * 128, c0:c1]
    return P


def _nzpairs(M):
    out = []
    for ob in range(3):
        for kc in range(3):
            if np.any(M[kc * 128:(kc + 1) * 128, ob * 128:(ob + 1) * 128]):
                out.append((kc, ob))
    return out


def _consts():
    c = {}
    Msm = _band(np.array([1, 2, 1], np.float32) / 8.0, "edge")
    Mdf = _band(np.array([-1, 0, 1], np.float32), "edge")
    Mga = _band(_gauss7(), "reflect")
    coef = np.array([0.299, 0.587, 0.114], np.float32)
    b1s = np.concatenate([coef[i] * Msm for i in range(3)], axis=0)
    b1d = np.concatenate([coef[i] * Mdf for i in range(3)], axis=0)
    c["w1s"] = _wins(b1s, 9)
    c["w1d"] = _wins(b1d, 9)
    c["wga3"] = _wins(Mga, 3)
    c["pr_df"] = _nzpairs(Mdf)
    c["pr_sm"] = _nzpairs(Msm)
    c["pr_ga"] = _nzpairs(Mga)
    c["b1sp"] = _bf(_pack(b1s, 9, c["w1s"]))
    c["b1dp"] = _bf(_pack(b1d, 9, c["w1d"]))
    c["mgap"] = _bf(_pack(Mga, 3, c["wga3"]))
    c["msm"] = _bf(Msm)
    c["mdf"] = _bf(Mdf)
    c["mga"] = _bf(Mga)
    c["mgan"] = _bf(-Mga)
    S8 = np.zeros((128, 16), np.float32)
    S8[np.arange(16) * 8, np.arange(16)] = 1.0
    c["s8"] = _bf(S8)
    T16 = np.zeros((16, 128), np.float32)
    T16[np.arange(128) // 8, np.arange(128)] = 1.0
    c["t16"] = _bf(T16)
    c["id2"] = np.eye(2, dtype=np.float32)
    c["ninfh"] = _bf(np.full((128, 384), NEG, np.float32))
    c["lw0"] = np.array([[0.0, 0.25 / 64.0], [0.0, 0.25 / 64.0]], np.float32)
    halves = np.zeros((2, 128), np.float32)
    halves[0, :64] = 1.0
    halves[1, 64:] = 1.0
    c["e2b"] = halves.copy()
    c["e64"] = np.ascontiguousarray(halves.T)
    c["iota128"] = (np.arange(128, dtype=np.float32) % 64).reshape(128, 1)
    c["thrW0"] = (np.arange(64, dtype=np.float32) * (0.25 / 64.0)).reshape(64, 1)
    c["thrW1"] = (np.arange(64, dtype=np.float32) * (0.25 / 4096.0)).reshape(64, 1)
    c["ones128"] = np.ones((128, 1), np.float32)
    c["ones64h"] = _bf(np.ones((1, 64), np.float32))
    c["ones1"] = np.ones((1, 128), np.float32)
    return c


def build_program():
    C = _consts()
    nc = bacc.Bacc()

    imgs_d = nc.dram_tensor("imgs", [NIMG, 3, H, W], BF16, kind="ExternalInput")
    sd_d = nc.dram_tensor("sd", [NIMG, H, W], BF16, kind="ExternalInput")
    dsl_d = nc.dram_tensor("dsl", [NIMG, 128, 2, NDESC], FP8, kind="ExternalInput")
    dnm_d = nc.dram_tensor("dnm", [NIMG, 128, 16, DDIM], FP8, kind="ExternalInput")
    b1sp_d = nc.dram_tensor("b1sp", [1152, BW], BF16, kind="ExternalInput")
    b1dp_d = nc.dram_tensor("b1dp", [1152, BW], BF16, kind="ExternalInput")
    mgap_d = nc.dram_tensor("mgap", [384, BW], BF16, kind="ExternalInput")
    msm_d = nc.dram_tensor("msm", [384, 384], BF16, kind="ExternalInput")
    mdf_d = nc.dram_tensor("mdf", [384, 384], BF16, kind="ExternalInput")
    mga_d = nc.dram_tensor("mga", [384, 384], BF16, kind="ExternalInput")
    mgan_d = nc.dram_tensor("mgan", [384, 384], BF16, kind="ExternalInput")
    s8_d = nc.dram_tensor("s8", [128, 16], BF16, kind="ExternalInput")
    t16_d = nc.dram_tensor("t16", [16, 128], BF16, kind="ExternalInput")
    id2_d = nc.dram_tensor("id2", [2, 2], F32, kind="ExternalInput")
    ninfh_d = nc.dram_tensor("ninfh", [128, 384], BF16, kind="ExternalInput")
    lw0_d = nc.dram_tensor("lw0", [2, 2], F32, kind="ExternalInput")
    e2b_d = nc.dram_tensor("e2b", [2, 128], F32, kind="ExternalInput")
    e64_d = nc.dram_tensor("e64", [128, 2], F32, kind="ExternalInput")
    iota128_d = nc.dram_tensor("iota128", [128, 1], F32, kind="ExternalInput")
    thrW0_d = nc.dram_tensor("thrW0", [64, 1], F32, kind="ExternalInput")
    thrW1_d = nc.dram_tensor("thrW1", [64, 1], F32, kind="ExternalInput")
    ones128_d = nc.dram_tensor("ones128", [128, 1], F32, kind="ExternalInput")
    ones1_d = nc.dram_tensor("ones1", [1, 128], F32, kind="ExternalInput")
    ones64h_d = nc.dram_tensor("ones64h", [1, 64], BF16, kind="ExternalInput")
    out_d = nc.dram_tensor("out", [4, 1], F32, kind="ExternalOutput")

    w1s, w1d, wga3 = C["w1s"], C["w1d"], C["wga3"]
    pr_df, pr_sm, pr_ga = C["pr_df"], C["pr_sm"], C["pr_ga"]

    with TileContext(nc) as tc:
        sb = tc.alloc_tile_pool(name="sb", bufs=1)
        sbi = tc.alloc_tile_pool(name="sbi", bufs=2)
        ps_cv = tc.alloc_tile_pool(name="pscv", bufs=2, space="PSUM")
        ps_g = tc.alloc_tile_pool(name="psg", bufs=2, space="PSUM")
        ps_m = tc.alloc_tile_pool(name="psm", bufs=2, space="PSUM")

        # ---- persistent SBUF ----
        b1sp_t = sb.tile([128, 9, BW], BF16)
        b1dp_t = sb.tile([128, 9, BW], BF16)
        mgap_t = sb.tile([128, 3, BW], BF16)
        msm_t = sb.tile([128, 3, 384], BF16)
        mdf_t = sb.tile([128, 3, 384], BF16)
        mga_t = sb.tile([128, 3, 384], BF16)
        mgan_t = sb.tile([128, 3, 384], BF16)
        s8_t = sb.tile([128, 16], BF16)
        t16_t = sb.tile([16, 128], BF16)
        id2_t = sb.tile([2, 2], F32)
        ninfh_t = sb.tile([128, 384], BF16)
        lw0_t = sb.tile([2, 2], F32)
        e2b_t = sb.tile([2, 128], F32)
        e64_t = sb.tile([128, 2], F32)
        iota128_t = sb.tile([128, 1], F32)
        thrW0_t = sb.tile([64, 1], F32)
        thrW1_t = sb.tile([64, 1], F32)
        ones128_t = sb.tile([128, 1], F32)
        ones1_t = sb.tile([1, 128], F32)
        ones64h_t = sb.tile([1, 64], BF16)

        spacc = sb.tile([128, 2], F32)     # softplus accums per image
        dacc = sb.tile([128, 6], F32)      # dot accums per (image, cc)
        gall = sb.tile([128, 4], F32)      # TTR accums: cross0, diag0, cross1, diag1
        xrow0_t = sb.tile([1, 2304], BF16)
        xrow1_t = sb.tile([1, 2304], BF16)
        xrow_tiles = [xrow0_t, xrow1_t]
        trash = sb.tile([128, 2304], BF16)
        trashf = sb.tile([128, 1216], F32)

        # colsum psum: [128, 512]: col = b*256 + t*16 + bi (diag at bi==t)
        cs = ps_m.tile([128, 512], F32, tag="cs", bufs=1)
        nc.vector.memset(cs, 0.0)

        # ---- input DMAs ----
        # img0 first on sync; desc on gpsimd (pool idle early)
        img_tiles, sd_tiles, dsl_tiles, dnm_tiles = [], [], [], []
        img0 = sbi.tile([128, 3, 3, 384], BF16, tag="img", bufs=2)
        nc.sync.dma_start(out=img0,
                          in_=imgs_d[0].rearrange("c (hc p) w -> p c hc w", p=128))
        nc.sync.dma_start(out=b1sp_t,
                          in_=b1sp_d[:, :].rearrange("(k p) h -> p k h", p=128))
        nc.sync.dma_start(out=b1dp_t,
                          in_=b1dp_d[:, :].rearrange("(k p) h -> p k h", p=128))
        nc.sync.dma_start(out=msm_t, in_=msm_d[:, :].rearrange("(k p) h -> p k h", p=128))
        nc.sync.dma_start(out=mdf_t, in_=mdf_d[:, :].rearrange("(k p) h -> p k h", p=128))
        nc.sync.dma_start(out=mgap_t, in_=mgap_d[:, :].rearrange("(k p) h -> p k h", p=128))
        nc.sync.dma_start(out=mga_t, in_=mga_d[:, :].rearrange("(k p) h -> p k h", p=128))
        nc.sync.dma_start(out=mgan_t, in_=mgan_d[:, :].rearrange("(k p) h -> p k h", p=128))
        for b in range(NIMG):
            dnm = sbi.tile([128, 16, DDIM], FP8, tag="dnm", bufs=2)
            nc.gpsimd.dma_start(out=dnm, in_=dnm_d[b])
            dnm_tiles.append(dnm)
            dsl = sbi.tile([128, 2, NDESC], FP8, tag="dsl", bufs=2)
            nc.gpsimd.dma_start(out=dsl, in_=dsl_d[b])
            dsl_tiles.append(dsl)
        for b in range(NIMG):
            sdt = sbi.tile([128, 3, 384], BF16, tag="sdt", bufs=2)
            nc.gpsimd.dma_start(out=sdt,
                                in_=sd_d[b].rearrange("(c p) w -> p c w", p=128))
            sd_tiles.append(sdt)
        img1 = sbi.tile([128, 3, 3, 384], BF16, tag="img", bufs=2)
        nc.sync.dma_start(out=img1,
                          in_=imgs_d[1].rearrange("c (hc p) w -> p c hc w", p=128))
        img_tiles.extend([img0, img1])
        for t, d in [(s8_t, s8_d), (t16_t, t16_d), (id2_t, id2_d), (ninfh_t, ninfh_d),
                     (lw0_t, lw0_d), (e2b_t, e2b_d), (e64_t, e64_d),
                     (iota128_t, iota128_d), (thrW0_t, thrW0_d), (thrW1_t, thrW1_d),
                     (ones128_t, ones128_d), (ones1_t, ones1_d),
                     (ones64h_t, ones64h_d)]:
            nc.sync.dma_start(out=t, in_=d[:, :])

        # ---- r = rsqrt(nsq) from n-major fp8: wide square + wide reduce ----
        r_tiles = []
        dsq = sb.tile([128, 16, DDIM], BF16)

        def emit_r(b):
            dnm = dnm_tiles[b]
            nc.scalar.activation(dsq, dnm, AF.Square)
            nsqf = sbi.tile([128, 16], F32, tag="nsqf", bufs=2)
            nc.vector.tensor_reduce(nsqf, dsq, axis=AX.X, op=ALU.add)
            sr = sbi.tile([128, 16], F32, tag="sr", bufs=2)
            nc.scalar.activation(sr, nsqf, AF.Sqrt)
            y0 = sbi.tile([128, 16], F32, tag="y0", bufs=2)
            nc.vector.reciprocal(y0, sr)
            yy = sbi.tile([128, 16], F32, tag="yy", bufs=2)
            nc.vector.tensor_tensor(out=yy, in0=y0, in1=y0, op=ALU.mult)
            nc.vector.tensor_tensor(out=yy, in0=yy, in1=nsqf, op=ALU.mult)
            nc.vector.tensor_scalar(yy, yy, -0.5, 1.5, op0=ALU.mult, op1=ALU.add)
            r_bf = sbi.tile([128, 16], BF16, tag="rbf", bufs=2)
            nc.vector.tensor_tensor(out=r_bf, in0=yy, in1=y0, op=ALU.mult)
            r_tiles.append(r_bf)

        # ---- gram tile generator ----
        def gram_tiles_gen():
            for b in range(NIMG):
                for bi in range(16):
                    c0 = 128 * bi
                    pos = c0
                    while pos < NDESC:
                        wdt = min(1024, NDESC - pos)
                        yield (b, bi, pos, wdt)
                        pos += wdt

        _gram_iter = gram_tiles_gen()
        _gram_state = {"done": False, "acc": 0.0, "pending": []}

        def _emit_matvecs(ent):
            grelu, b, bi, pos, wdt = ent
            r_bf = r_tiles[b]
            for ci in range(wdt // 128):
                t = (pos + ci * 128) // 128
                col = 256 * b + 16 * t + bi
                nc.tensor.matmul(cs[:, col:col + 1],
                                 grelu[:, 128 * ci:128 * (ci + 1)],
                                 r_bf[:, bi:bi + 1],
                                 start=True, stop=True)

        def pump_gram(n, act_share=0.0):
            for _ in range(n):
                if _gram_state["done"]:
                    break
                try:
                    b, bi, pos, wdt = next(_gram_iter)
                except StopIteration:
                    _gram_state["done"] = True
                    break
                dsl = dsl_tiles[b]
                gp = ps_g.tile([128, 1024], F32, tag="g")
                off = 0
                while off < wdt:
                    nn = min(512, wdt - off)
                    nc.tensor.matmul(gp[:, off:off + nn],
                                     dsl[:, :, 128 * bi:128 * (bi + 1)],
                                     dsl[:, :, pos + off:pos + off + nn],
                                     start=True, stop=True, perf_mode=DR)
                    off += nn
                grelu = sbi.tile([128, 1024], BF16, tag="grelu", bufs=5)
                _gram_state["acc"] += act_share
                if _gram_state["acc"] >= 1.0:
                    _gram_state["acc"] -= 1.0
                    nc.scalar.activation(grelu[:, 0:wdt], gp[:, 0:wdt], AF.Relu)
                else:
                    nc.vector.tensor_scalar(grelu[:, 0:wdt], gp[:, 0:wdt], 0.0,
                                            None, op0=ALU.max)
                _gram_state["pending"].append((grelu, b, bi, pos, wdt))
                while len(_gram_state["pending"]) > 3:
                    _emit_matvecs(_gram_state["pending"].pop(0))
            if _gram_state["done"]:
                while _gram_state["pending"]:
                    _emit_matvecs(_gram_state["pending"].pop(0))

        # ---- conv + NMS per image ----
        resp_list, nms_list, bw_list = [], [], []

        def emit_conv(b, pump=True):
            img_t = img_tiles[b]
            imgv = img_t.rearrange("p c hc w -> p (c hc) w")

            # P1: smooth/diff along H -> [w-part, wb, h]
            sT = sbi.tile([128, 3, 384], BF16, tag="sT")
            dT = sbi.tile([128, 3, 384], BF16, tag="dT")
            for di, (dst, bnd, wins) in enumerate(
                    ((sT, b1sp_t, w1s), (dT, b1dp_t, w1d))):
                for wb in range(3):
                    pst = ps_cv.tile([128, 384], F32, tag="cv")
                    first = True
                    for k in range(9):
                        if wins[k] is None:
                            continue
                        c0, c1 = wins[k]
                        nc.tensor.matmul(pst[:, c0:c1],
                                         imgv[:, k, wb * 128:(wb + 1) * 128],
                                         bnd[:, k, 0:c1 - c0], start=first, stop=False)
                        first = False
                    if (di * 3 + wb) % 2 == 0:
                        nc.scalar.copy(dst[:, wb, :], pst)
                    else:
                        nc.vector.tensor_copy(dst[:, wb, :], pst)

            # P2: diff/smooth along W -> px=dx, py=dy [w-part(ob), h]
            pq = sbi.tile([128, 3, 384], BF16, tag="pq", bufs=1)
            qq = sbi.tile([128, 3, 384], BF16, tag="qq", bufs=1)
            rr = sbi.tile([128, 3, 384], BF16, tag="rr", bufs=1)
            dxs = sbi.tile([128, 384], BF16, tag="dxs", bufs=2)
            for ob in range(3):
                px = ps_cv.tile([128, 384], F32, tag="cv")
                fx = True
                for kc, ob2 in pr_df:
                    if ob2 != ob:
                        continue
                    nc.tensor.matmul(px, mdf_t[:, kc, ob * 128:(ob + 1) * 128],
                                     sT[:, kc, :], start=fx, stop=False)
                    fx = False
                nc.scalar.activation(pq[:, ob, :], px, AF.Square)
                nc.scalar.copy(dxs, px)
                py = ps_cv.tile([128, 384], F32, tag="cv")
                fy = True
                for kc, ob2 in pr_sm:
                    if ob2 != ob:
                        continue
                    nc.tensor.matmul(py, msm_t[:, kc, ob * 128:(ob + 1) * 128],
                                     dT[:, kc, :], start=fy, stop=False)
                    fy = False
                nc.scalar.activation(qq[:, ob, :], py, AF.Square)
                nc.vector.tensor_tensor(out=rr[:, ob, :], in0=dxs, in1=py, op=ALU.mult)
            if pump:
                pump_gram(2, act_share=0.2)

            # P3: gauss along W -> [h-part(hb), w]
            gP = sbi.tile([128, 3, 384], BF16, tag="gP", bufs=1)
            gM = sbi.tile([128, 3, 384], BF16, tag="gM", bufs=1)
            gR = sbi.tile([128, 3, 384], BF16, tag="gR", bufs=1)
            for si, (src, dst) in enumerate(((pq, gP), (qq, gM), (rr, gR))):
                for hb in range(3):
                    pst = ps_cv.tile([128, 384], F32, tag="cv")
                    for i, cw in enumerate(range(3)):
                        c0, c1 = wga3[cw]
                        nc.tensor.matmul(pst[:, c0:c1],
                                         src[:, cw, hb * 128:(hb + 1) * 128],
                                         mgap_t[:, cw, 0:c1 - c0],
                                         start=(i == 0), stop=False)
                    if (si * 3 + hb) % 3 == 0:
                        nc.vector.tensor_copy(dst[:, hb, :], pst)
                    else:
                        nc.scalar.copy(dst[:, hb, :], pst)

            if pump:
                pump_gram(2, act_share=0.2)

            # P4: gauss along H -> P (tr), M, R in [h-part(ob), w]; response
            resp = sbi.tile([128, 3, 388], BF16, tag="resp", bufs=2)
            for ob in range(3):
                pR = ps_cv.tile([128, 384], F32, tag="cv")
                first = True
                for kc, ob2 in pr_ga:
                    if ob2 != ob:
                        continue
                    nc.tensor.matmul(pR, mga_t[:, kc, ob * 128:(ob + 1) * 128],
                                     gR[:, kc, :], start=first, stop=False)
                    first = False
                B4 = sbi.tile([128, 384], BF16, tag="B4", bufs=2)
                nc.scalar.activation(B4, pR, AF.Square, scale=2.0)
                pM = ps_cv.tile([128, 384], F32, tag="cv")
                first = True
                for kc, ob2 in pr_ga:
                    if ob2 != ob:
                        continue
                    nc.tensor.matmul(pM, mga_t[:, kc, ob * 128:(ob + 1) * 128],
                                     gP[:, kc, :], start=first, stop=False)
                    first = False
                for kc, ob2 in pr_ga:
                    if ob2 != ob:
                        continue
                    nc.tensor.matmul(pM, mgan_t[:, kc, ob * 128:(ob + 1) * 128],
                                     gM[:, kc, :], start=False, stop=False)
                A2 = sbi.tile([128, 384], BF16, tag="A2", bufs=2)
                nc.scalar.activation(A2, pM, AF.Square)
                disc = sbi.tile([128, 384], BF16, tag="disc", bufs=2)
                nc.vector.tensor_tensor(out=disc, in0=A2, in1=B4, op=ALU.add)
                s2 = sbi.tile([128, 384], F32, tag="s2", bufs=2)
                nc.scalar.activation(s2, disc, AF.Sqrt, scale=0.25)
                pP = ps_cv.tile([128, 384], F32, tag="cv")
                first = True
                for kc, ob2 in pr_ga:
                    if ob2 != ob:
                        continue
                    nc.tensor.matmul(pP, mga_t[:, kc, ob * 128:(ob + 1) * 128],
                                     gP[:, kc, :], start=first, stop=False)
                    first = False
                for kc, ob2 in pr_ga:
                    if ob2 != ob:
                        continue
                    nc.tensor.matmul(pP, mga_t[:, kc, ob * 128:(ob + 1) * 128],
                                     gM[:, kc, :], start=False, stop=False)
                nc.vector.scalar_tensor_tensor(out=resp[:, ob, 2:386], in0=pP,
                                               scalar=0.5, in1=s2,
                                               op0=ALU.mult, op1=ALU.subtract)
                nc.vector.tensor_copy(resp[:, ob, 0:2], ninfh_t[:, 0:2])
                nc.vector.tensor_copy(resp[:, ob, 386:388], ninfh_t[:, 0:2])
            resp_list.append(resp)

        def emit_nms_gen(b):
            TTs = nc.vector.tensor_tensor
            resp = resp_list[b]
            sdt = sd_tiles[b]
            sdv = sdt.rearrange("p c w -> p (c w)")
            spA = sbi.tile([128, 1152], F32, tag="spA", bufs=1)
            nc.scalar.activation(spA, sdv, AF.Exp)
            nc.scalar.activation(trashf[:, 0:1152], spA, AF.Ln, bias=1.0,
                                 accum_out=spacc[:, b:b + 1])

            t1 = sbi.tile([128, 3, 388], BF16, tag="t1", bufs=1)
            TTs(out=t1[:, :, 0:387], in0=resp[:, :, 0:387],
                in1=resp[:, :, 1:388], op=ALU.max)
            t2 = sbi.tile([128, 3, 388], BF16, tag="t2", bufs=1)
            TTs(out=t2[:, :, 0:385], in0=t1[:, :, 0:385],
                in1=t1[:, :, 2:387], op=ALU.max)
            m1 = sbi.tile([128, 3, 384], BF16, tag="m1", bufs=2)
            TTs(out=m1, in0=t2[:, :, 0:384], in1=resp[:, :, 4:388], op=ALU.max)
            yield
            shs = []
            for k in (1, 2):
                sh = sbi.tile([128, 3, 384], BF16, tag="shd", bufs=2)
                nc.vector.memset(sh[0:k, 0, :], NEG)
                nc.sync.dma_start(out=sh[k:128], in_=m1[0:128 - k])
                nc.sync.dma_start(out=sh[0:k, 1:3, :], in_=m1[128 - k:128, 0:2, :])
                shs.append(sh)
                sh2 = sbi.tile([128, 3, 384], BF16, tag="shu", bufs=2)
                nc.gpsimd.dma_start(out=sh2[128 - k:128, 2, :],
                                    in_=ninfh_t[0:k, 0:384])
                nc.gpsimd.dma_start(out=sh2[0:128 - k], in_=m1[k:128])
                nc.gpsimd.dma_start(out=sh2[128 - k:128, 0:2, :], in_=m1[0:k, 1:3, :])
                shs.append(sh2)
            yield
            mp1 = sbi.tile([128, 3, 384], BF16, tag="mp1", bufs=1)
            nc.vector.tensor_tensor(out=mp1, in0=m1, in1=shs[0], op=ALU.max)
            mp2 = sbi.tile([128, 3, 384], BF16, tag="mp2", bufs=1)
            TTs(out=mp2, in0=shs[1], in1=shs[2], op=ALU.max)
            nc.vector.tensor_tensor(out=mp2, in0=mp2, in1=shs[3], op=ALU.max)
            mp = sbi.tile([128, 3, 384], BF16, tag="mp", bufs=2)
            nc.vector.tensor_tensor(out=mp, in0=mp1, in1=mp2, op=ALU.max)
            yield
            e1 = sbi.tile([128, 3, 384], BF16, tag="e1", bufs=1)
            TTs(out=e1, in0=resp[:, :, 2:386], in1=mp, op=ALU.is_ge)
            nms = sbi.tile([128, 3, 384], BF16, tag="nms", bufs=2)
            nc.vector.tensor_tensor(out=nms, in0=resp[:, :, 2:386], in1=e1, op=ALU.mult)
            nms_list.append(nms)
            yield
            bw = sbi.tile([128, 3, 48], BF16, tag="bw")
            nc.vector.tensor_reduce(bw, nms.rearrange("p c (g k) -> p c g k", k=8),
                                    axis=AX.X, op=ALU.max)
            shbs = []
            for k in range(1, 8):
                shk = sbi.tile([128, 3, 48], BF16, tag="shb", bufs=8)
                q = nc.sync if k % 2 else nc.gpsimd
                q.dma_start(out=shk[0:121], in_=bw[k:121 + k])
                shbs.append(shk)
            yield
            q1 = sbi.tile([128, 3, 48], BF16, tag="bwm", bufs=2)
            nc.vector.tensor_tensor(out=q1[0:121], in0=bw[0:121], in1=shbs[0][0:121],
                                    op=ALU.max)
            q2 = sbi.tile([128, 3, 48], BF16, tag="bwm", bufs=2)
            nc.vector.tensor_tensor(out=q2[0:121], in0=shbs[1][0:121],
                                    in1=shbs[2][0:121], op=ALU.max)
            q3 = sbi.tile([128, 3, 48], BF16, tag="bwm3", bufs=2)
            nc.vector.tensor_tensor(out=q3[0:121], in0=shbs[3][0:121],
                                    in1=shbs[4][0:121], op=ALU.max)
            q4 = sbi.tile([128, 3, 48], BF16, tag="bwm3", bufs=2)
            nc.vector.tensor_tensor(out=q4[0:121], in0=shbs[5][0:121],
                                    in1=shbs[6][0:121], op=ALU.max)
            q5 = sbi.tile([128, 3, 48], BF16, tag="bwm5", bufs=2)
            nc.vector.tensor_tensor(out=q5[0:121], in0=q1[0:121], in1=q2[0:121],
                                    op=ALU.max)
            q6 = sbi.tile([128, 3, 48], BF16, tag="bwm5", bufs=2)
            nc.vector.tensor_tensor(out=q6[0:121], in0=q3[0:121], in1=q4[0:121],
                                    op=ALU.max)
            yield
            cur = sbi.tile([128, 3, 48], BF16, tag="bwf", bufs=2)
            nc.vector.tensor_tensor(out=cur[0:121], in0=q5[0:121], in1=q6[0:121],
                                    op=ALU.max)
            yield
            p16 = ps_m.tile([16, 144], F32, tag="m", bufs=1)
            nc.tensor.matmul(p16, s8_t[0:121, :],
                             cur[0:121].rearrange("p c g -> p (c g)"),
                             start=True, stop=True)
            p16s = sbi.tile([16, 3, 48], BF16, tag="p16s", bufs=2)
            nc.scalar.copy(p16s.rearrange("p c g -> p (c g)"), p16)
            bw_list.append(p16s)
            xr = xrow_tiles[b]
            p16f = p16s.rearrange("p c g -> p (c g)")
            nc.sync.dma_start(out=xr[0:1, 0:1152], in_=p16f[0:8, :])
            nc.gpsimd.dma_start(out=xr[0:1, 1152:2304], in_=p16f[8:16, :])

        # ---- per-image threshold search + selection (interleavable) ----
        W0 = 0.25 / 64.0
        W1 = 0.25 / 4096.0
        maskz = sb.tile([128, 2, 2], F32)   # [*, img, round] zero-padded masks
        nc.vector.memset(maskz, 0.0)
        trash64 = sb.tile([64, 2304], BF16)

        def thresh_sel_gen_img(b):
            xr = xrow_tiles[b]
            x64 = sbi.tile([64, 2304], BF16, tag="x64", bufs=2)
            nc.gpsimd.partition_broadcast(x64, xr, channels=64)
            yield
            # round 1 (constant thresholds)
            cnt = sbi.tile([64, 1], F32, tag="cnt", bufs=2)
            nc.vector.tensor_scalar(trash64, x64, thrW0_t[:, 0:1], None,
                                    op0=ALU.is_gt, op1=ALU.add, accum_out=cnt)
            nc.vector.tensor_scalar(maskz[0:64, b, 0:1], cnt, float(NUM) - 0.5,
                                    None, op0=ALU.is_ge)
            yield
            kp1 = ps_m.tile([1, 1], F32, tag="m", bufs=1)
            nc.tensor.matmul(kp1, maskz[:, b, 0:1], ones128_t, start=True, stop=True)
            lo_sc = sbi.tile([1, 1], F32, tag="losc", bufs=2)
            nc.vector.tensor_scalar(lo_sc, kp1, W0, -W0, op0=ALU.mult, op1=ALU.add)
            lo64p = ps_m.tile([64, 1], F32, tag="m", bufs=1)
            nc.tensor.matmul(lo64p, ones1[0:1, 0:64] if False else ones1_t[0:1, 0:64],
                             lo_sc, start=True, stop=True)
            T2 = sbi.tile([64, 1], F32, tag="T2", bufs=2)
            nc.vector.tensor_tensor(out=T2, in0=lo64p, in1=thrW1_t, op=ALU.add)
            yield
            # round 2
            cnt2 = sbi.tile([64, 1], F32, tag="cnt2", bufs=2)
            nc.vector.tensor_scalar(trash64, x64, T2[:, 0:1], None,
                                    op0=ALU.is_gt, op1=ALU.add, accum_out=cnt2)
            nc.vector.tensor_scalar(maskz[0:64, b, 1:2], cnt2, float(NUM) - 0.5,
                                    None, op0=ALU.is_ge)
            yield
            kp2 = ps_m.tile([1, 1], F32, tag="m", bufs=1)
            nc.tensor.matmul(kp2, maskz[:, b, 1:2], ones128_t, start=True, stop=True)
            tf = sbi.tile([1, 1], F32, tag="tf", bufs=2)
            nc.vector.tensor_scalar(tf, kp2, W1, -W1, op0=ALU.mult, op1=ALU.add)
            nc.vector.tensor_tensor(out=tf, in0=tf, in1=lo_sc, op=ALU.add)
            nc.vector.tensor_scalar(tf, tf, 1e-30, None, op0=ALU.max)
            tb16p = ps_m.tile([16, 1], F32, tag="m", bufs=1)
            nc.tensor.matmul(tb16p, ones1_t[0:1, 0:16], tf, start=True, stop=True)
            tb16 = sbi.tile([16, 1], F32, tag="tb16", bufs=2)
            nc.scalar.copy(tb16, tb16p)
            yield
            # selection + dot
            p16s = bw_list[b]
            nms = nms_list[b]
            sdt = sd_tiles[b]
            p16c = sbi.tile([16, 3, 48], BF16, tag="p16c", bufs=2)
            nc.vector.tensor_scalar(p16c.rearrange("p c g -> p (c g)"),
                                    p16s.rearrange("p c g -> p (c g)"),
                                    tb16[:, 0:1], None, op0=ALU.max)
            yield
            for cc in range(3):
                bexp = ps_m.tile([128, 384], F32, tag="m", bufs=1)
                nc.tensor.matmul(bexp, t16_t,
                                 p16c[:, cc, :].unsqueeze(2)
                                 .to_broadcast([16, 48, 8]),
                                 start=True, stop=True)
                sel = sbi.tile([128, 384], BF16, tag="sel", bufs=2)
                nc.vector.tensor_tensor(out=sel, in0=nms[:, cc, :], in1=bexp,
                                        op=ALU.is_ge)
                dtmp = sbi.tile([128, 384], BF16, tag="dtmp", bufs=2)
                nc.vector.scalar_tensor_tensor(
                    out=dtmp, in0=sel, scalar=1.0, in1=sdt[:, cc, :],
                    op0=ALU.mult, op1=ALU.mult,
                    accum_out=dacc[:, 3 * b + cc:3 * b + cc + 1])
                yield

        # ================= schedule =================
        def drive(gen, tiles_per_step=1, act_share=0.45):
            for _ in gen:
                pump_gram(tiles_per_step, act_share=act_share)

        emit_conv(0, pump=False)
        pump_gram(3)
        emit_r(0)
        emit_r(1)
        drive(emit_nms_gen(0), 1, 0.3)
        emit_conv(1)
        drive(thresh_sel_gen_img(0), 0, 0.4)
        drive(emit_nms_gen(1), 2, 0.7)
        drive(thresh_sel_gen_img(1), 2, 0.7)
        pump_gram(1000, act_share=0.7)

        # gram weighted sums per image: full (incl diag) and diag-only
        for b in range(NIMG):
            wcs = sbi.tile([128, 256], F32, tag="wcs", bufs=2)
            nc.vector.tensor_tensor(
                out=wcs.rearrange("p (a k) -> p a k", k=16),
                in0=cs[:, 256 * b:256 * b + 256].rearrange("p (a k) -> p a k", k=16),
                in1=r_tiles[b].unsqueeze(2).to_broadcast([128, 16, 16]),
                op=ALU.mult)
            nc.vector.tensor_reduce(gall[:, 2 * b:2 * b + 1], wcs,
                                    axis=AX.X, op=ALU.add)
            wcd = sbi.tile([128, 16], F32, tag="wcd", bufs=2)
            import concourse.ap as ap_mod
            csap = cs[:, 256 * b:256 * b + 256]
            diag_ap = ap_mod.AP(csap.tensor, csap.offset,
                                [list(csap.ap[0]), [17 * csap.ap[1][0], 16]])
            nc.vector.tensor_tensor(out=wcd, in0=diag_ap, in1=r_tiles[b],
                                    op=ALU.mult)
            nc.vector.tensor_reduce(gall[:, 2 * b + 1:2 * b + 2], wcd,
                                    axis=AX.X, op=ALU.add)

        # ---- final reduce ----
        vals = sb.tile([128, 4], F32)
        nc.vector.tensor_reduce(vals[:, 0:1], spacc, axis=AX.X, op=ALU.add)
        nc.vector.tensor_reduce(vals[:, 1:2], dacc, axis=AX.X, op=ALU.add)
        nc.vector.tensor_tensor(out=vals[:, 2:3], in0=gall[:, 0:1], in1=gall[:, 2:3],
                                op=ALU.add)
        nc.vector.tensor_tensor(out=vals[:, 3:4], in0=gall[:, 1:2], in1=gall[:, 3:4],
                                op=ALU.add)
        fps = ps_m.tile([4, 1], F32, tag="m", bufs=1)
        nc.tensor.matmul(fps, vals, ones128_t, start=True, stop=True)
        fsb = sb.tile([4, 1], F32)
        nc.scalar.copy(fsb, fps)
        nc.sync.dma_start(out=out_d[:, :], in_=fsb)

        ps_m.release()
        ps_g.release()
        ps_cv.release()
        sbi.release()
        sb.release()

    nc.finalize()
    return nc, C


_CACHE = {}


def kernel(descriptors, scores, scores_dense, imgs):
    B = descriptors.shape[0]
    ncore = 8
    per = B // ncore
    if "nc" not in _CACHE:
        _CACHE["nc"], _CACHE["C"] = build_program()
    nc, C = _CACHE["nc"], _CACHE["C"]

    imgs_bf = np.ascontiguousarray(np.asarray(imgs).astype(ml_dtypes.bfloat16))
    sd = np.ascontiguousarray(np.asarray(scores_dense).reshape(B, H, W)
                              .astype(ml_dtypes.bfloat16))
    desc8 = np.asarray(descriptors).astype(ml_dtypes.float8_e4m3)
    # slab d-major: [B, 128(dj), 2(slab), N]
    dsl = np.ascontiguousarray(desc8.transpose(0, 2, 1)
                               .reshape(B, 2, 128, NDESC).transpose(0, 2, 1, 3))
    # n-major: [B, 128(n in chunk), 16(chunk), D]
    dnm = np.ascontiguousarray(desc8.reshape(B, 16, 128, DDIM).transpose(0, 2, 1, 3))

    in_maps = []
    for c in range(ncore):
        s = slice(c * per, (c + 1) * per)
        in_maps.append({
            "imgs": imgs_bf[s], "sd": sd[s], "dsl": dsl[s], "dnm": dnm[s],
            "b1sp": C["b1sp"], "b1dp": C["b1dp"], "mgap": C["mgap"],
            "msm": C["msm"], "mdf": C["mdf"], "mga": C["mga"], "mgan": C["mgan"],
            "s8": C["s8"], "t16": C["t16"], "id2": C["id2"], "ninfh": C["ninfh"],
            "lw0": C["lw0"], "e2b": C["e2b"], "e64": C["e64"],
            "iota128": C["iota128"], "thrW0": C["thrW0"], "thrW1": C["thrW1"],
            "ones128": C["ones128"], "ones1": C["ones1"],
            "ones64h": C["ones64h"],
        })

    res = run_bass_kernel_spmd(nc, in_maps, core_ids=list(range(ncore)))
    S1 = S2 = Sall = Sdia = 0.0
    for c in range(ncore):
        o = np.asarray(res.results[c]["out"])[:, 0].astype(np.float64)
        S1 += o[0]
        S2 += o[1]
        Sall += o[2]
        Sdia += o[3]
    bce = (S1 - S2) / (B * H * W)
    relu_mean = (2.0 * Sall - Sdia) / (B * NDESC * NDESC)
    return np.array(bce + relu_mean, dtype=np.float32)



# revision 57
# speedup vs baseline: 1.4916x; 1.4916x over previous
"""Trainium2 Bass kernel for nn_DistinctionLoss (GFTT corners BCE + relu-cosine mean).

v11: batch-sharded 2 images/core across 8 cores. 72.0us/core (sim), 1.49x vs v2.
 - Descriptors L2-normalized on host then cast e4m3: the fp8 DoubleRow gram IS
   the cosine matrix. Upper triangle packed into 1024-wide PSUM tiles (15
   strict + 2 diagonal-block tiles/image); one relu+row-accum per tile,
   statically balanced between Act and DVE (ENG_PATTERN). Host combines:
   full = 2*strict + diagblk; matrix diagonal folds into the constant N.
 - GFTT: gray/sobel/gauss as band matmuls; squared channels are (dx^2, dy^2,
   (dx+dy)^2) so dxdy never needs an elementwise multiply. PSUM chains are
   continued after extraction to reuse partial sums on PE: (dx+dy) reuses the
   dx psum, and pP = pM + 2*G(q2) reuses the pM psum.
 - 5x5 NMS: shifted-max trees on DVE (W by slicing, H by DMA partition shifts).
 - 8x8 block max: W-tree then PE-transpose so H lands on the free axis,
   tree-max, transpose back (no shift-DMA chain).
 - Top-200 threshold: single 128-grid count round (bcast via gpsimd, is_gt
   accum at DVE 4x), partition_all_reduce -> per-partition threshold; the
   final dot is one tiny [16,144] STT over block maxima * block sums
   (selection mask and csd=mask*sd precomputed off the critical path),
   accumulated straight into the output vector (no final dot reduce).
 - Tail DMAs (block-max rows -> threshold source) split into quarter-rows
   across SP/Act/Pool queues.
 - All Sqrt before first Exp/Ln -> exactly 2 act-table loads; softplus fused
   over both images.
 - All DMA inputs repacked partition-major on host; input DMAs spread across
   SP/Act/Pool queues; latency-critical img1 ops pinned off Pool.
"""
import os
import numpy as np
import ml_dtypes

import concourse.bacc as bacc
import concourse.bass_isa as bass_isa
import concourse.mybir as mybir
from concourse.tile import TileContext
from concourse.bass_utils import run_bass_kernel_spmd

F32 = mybir.dt.float32
BF16 = mybir.dt.bfloat16
FP8 = mybir.dt.float8e4
AF = mybir.ActivationFunctionType
ALU = mybir.AluOpType
AX = mybir.AxisListType
DR = mybir.MatmulPerfMode.DoubleRow

H = W = 384
NIMG = 2
NDESC = 2048
DDIM = 256
NUM = 200
NEG = -1e30
BW = 136  # packed band window width

_bf = lambda a: np.ascontiguousarray(a.astype(ml_dtypes.bfloat16))


def _band(k, mode, n=384):
    pad = len(k) // 2
    idx = np.arange(n + 2 * pad) - pad
    if mode == "edge":
        src = np.clip(idx, 0, n - 1)
    else:  # reflect
        src = np.abs(idx)
        src = np.where(src >= n, 2 * (n - 1) - src, src)
    M = np.zeros((n, n), np.float32)
    for i, kv in enumerate(k):
        M[src[np.arange(n) + i], np.arange(n)] += kv
    return M


def _gauss7():
    xs = np.arange(7, dtype=np.float32) - 3.0
    g = np.exp(-0.5 * xs ** 2)
    return (g / g.sum()).astype(np.float32)


def _wins(M, nchunk):
    wins = []
    for k in range(nchunk):
        rows = M[k * 128:(k + 1) * 128]
        nz = np.nonzero(np.any(rows != 0, axis=0))[0]
        wins.append((int(nz[0]), int(nz[-1]) + 1) if len(nz) else None)
    return wins


def _pack(M, nchunk, wins):
    """Pack band matrix rows into [nchunk*128, BW] windows."""
    P = np.zeros((nchunk * 128, BW), np.float32)
    for k in range(nchunk):
        if wins[k] is None:
            continue
        c0, c1 = wins[k]
        P[k * 128:(k + 1) * 128, 0:c1 - c0] = M[k * 128:(k + 1) * 128, c0:c1]
    return P


def _nzpairs(M):
    out = []
    for ob in range(3):
        for kc in range(3):
            if np.any(M[kc * 128:(kc + 1) * 128, ob * 128:(ob + 1) * 128]):
                out.append((kc, ob))
    return out


def _consts():
    c = {}
    Msm = _band(np.array([1, 2, 1], np.float32) / 8.0, "edge")
    Mdf = _band(np.array([-1, 0, 1], np.float32), "edge")
    Mga = _band(_gauss7(), "reflect")
    coef = np.array([0.299, 0.587, 0.114], np.float32)
    b1s = np.concatenate([coef[i] * Msm for i in range(3)], axis=0)
    b1d = np.concatenate([coef[i] * Mdf for i in range(3)], axis=0)
    c["w1s"] = _wins(b1s, 9)
    c["w1d"] = _wins(b1d, 9)
    c["wga3"] = _wins(Mga, 3)
    c["pr_df"] = _nzpairs(Mdf)
    c["pr_sm"] = _nzpairs(Msm)
    c["pr_ga"] = _nzpairs(Mga)
    pmaj = lambda M, k: np.ascontiguousarray(
        M.reshape(k, 128, -1).transpose(1, 0, 2))
    c["b1sp"] = _bf(pmaj(_pack(b1s, 9, c["w1s"]), 9))
    c["b1dp"] = _bf(pmaj(_pack(b1d, 9, c["w1d"]), 9))
    c["mgap"] = _bf(pmaj(_pack(Mga, 3, c["wga3"]), 3))
    c["msm"] = _bf(pmaj(Msm, 3))
    c["mdf"] = _bf(pmaj(Mdf, 3))
    c["mga"] = _bf(pmaj(Mga, 3))
    c["mgan"] = _bf(pmaj(-Mga, 3))
    c["mga2"] = _bf(pmaj(2.0 * Mga, 3))
    S8b = np.zeros((128, 16), np.float32)
    S8b[np.arange(128), np.arange(128) // 8] = 1.0
    c["s8b"] = _bf(S8b)
    c["ident"] = _bf(np.eye(128, dtype=np.float32))
    T16 = np.zeros((16, 128), np.float32)
    T16[np.arange(128) // 8, np.arange(128)] = 1.0
    c["t16"] = _bf(T16)
    c["ninfh"] = _bf(np.full((128, 384), NEG, np.float32))
    c["thr128"] = (np.arange(128, dtype=np.float32) * (0.25 / 128.0)).reshape(128, 1)
    c["ones128"] = np.ones((128, 1), np.float32)
    return c


# Gram tiling: per image, the 16 diagonal 128x128 blocks pack into 2 tiles
# (their relu sums need a separate accumulator: full = 2*strict + diagblk),
# and the strict upper triangle packs into exactly 15 1024-wide tiles.
def _gram_plan(width=1024):
    tiles = []
    for half in range(2):
        tiles.append(("diag", [(bi, 128 * bi, 128) for bi in
                               range(8 * half, 8 * half + 8)]))
    cur, fill = [], 0
    for bi in range(15):
        pos = 128 * (bi + 1)
        rem = NDESC - pos
        while rem > 0:
            take = min(rem, width - fill)
            cur.append((bi, pos, take))
            fill += take
            pos += take
            rem -= take
            if fill == width:
                tiles.append(("strict", cur))
                cur, fill = [], 0
    if cur:
        tiles.append(("strict", cur))
    return tiles


GRAM_TILES = _gram_plan()  # 17 tiles per image (2 diag + 15 strict)


# relu+accum engine per gram tile (34 total): 'A'=Activation, 'D'=DVE.
def _mk_pattern(na, nd, n=34):
    pat = []
    ca = cd = 0.0
    for i in range(n):
        da = (i + 1) * na / n - ca
        dd = (i + 1) * nd / n - cd
        if da >= dd:
            pat.append('A'); ca += 1
        else:
            pat.append('D'); cd += 1
    return pat


ENG_PATTERN = list('D'*17 + 'A'*10 + 'AAAAAAA')


def build_program():
    C = _consts()
    nc = bacc.Bacc()

    imgs_d = nc.dram_tensor("imgs", [NIMG, 128, 3, 3, 3, 128], BF16, kind="ExternalInput")
    sd_d = nc.dram_tensor("sd", [NIMG, 128, 3, 384], BF16, kind="ExternalInput")
    dsl_d = nc.dram_tensor("dsl", [NIMG, 128, 2, NDESC], FP8, kind="ExternalInput")
    b1sp_d = nc.dram_tensor("b1sp", [128, 9, BW], BF16, kind="ExternalInput")
    b1dp_d = nc.dram_tensor("b1dp", [128, 9, BW], BF16, kind="ExternalInput")
    mgap_d = nc.dram_tensor("mgap", [128, 3, BW], BF16, kind="ExternalInput")
    msm_d = nc.dram_tensor("msm", [128, 3, 384], BF16, kind="ExternalInput")
    mdf_d = nc.dram_tensor("mdf", [128, 3, 384], BF16, kind="ExternalInput")
    mga_d = nc.dram_tensor("mga", [128, 3, 384], BF16, kind="ExternalInput")
    mgan_d = nc.dram_tensor("mgan", [128, 3, 384], BF16, kind="ExternalInput")
    mga2_d = nc.dram_tensor("mga2", [128, 3, 384], BF16, kind="ExternalInput")
    s8b_d = nc.dram_tensor("s8b", [128, 16], BF16, kind="ExternalInput")
    ident_d = nc.dram_tensor("ident", [128, 128], BF16, kind="ExternalInput")
    t16_d = nc.dram_tensor("t16", [16, 128], BF16, kind="ExternalInput")
    ninfh_d = nc.dram_tensor("ninfh", [128, 384], BF16, kind="ExternalInput")
    thr128_d = nc.dram_tensor("thr128", [128, 1], F32, kind="ExternalInput")
    ones128_d = nc.dram_tensor("ones128", [128, 1], F32, kind="ExternalInput")
    out_d = nc.dram_tensor("out", [5, 1], F32, kind="ExternalOutput")

    w1s, w1d, wga3 = C["w1s"], C["w1d"], C["wga3"]
    pr_df, pr_sm, pr_ga = C["pr_df"], C["pr_sm"], C["pr_ga"]

    with TileContext(nc) as tc:
        sb = tc.alloc_tile_pool(name="sb", bufs=1)
        sbi = tc.alloc_tile_pool(name="sbi", bufs=2)
        ps_cv = tc.alloc_tile_pool(name="pscv", bufs=3, space="PSUM")
        ps_g = tc.alloc_tile_pool(name="psg", bufs=2, space="PSUM")
        ps_m = tc.alloc_tile_pool(name="psm", bufs=2, space="PSUM")

        # ---- persistent SBUF ----
        b1sp_t = sb.tile([128, 9, BW], BF16)
        b1dp_t = sb.tile([128, 9, BW], BF16)
        mgap_t = sb.tile([128, 3, BW], BF16)
        msm_t = sb.tile([128, 3, 384], BF16)
        mdf_t = sb.tile([128, 3, 384], BF16)
        mga_t = sb.tile([128, 3, 384], BF16)
        mgan_t = sb.tile([128, 3, 384], BF16)
        mga2_t = sb.tile([128, 3, 384], BF16)
        s8b_t = sb.tile([128, 16], BF16)
        ident_t = sb.tile([128, 128], BF16)
        t16_t = sb.tile([16, 128], BF16)
        ninfh_t = sb.tile([128, 384], BF16)
        thr128_t = sb.tile([128, 1], F32)
        ones128_t = sb.tile([128, 1], F32)

        spacc = sb.tile([128, 1], F32)     # softplus accum (both images)
        gracc = sb.tile([128, 48], F32)    # per-gram-tile relu row sums
        xrow_tiles = [[sb.tile([1, 1152], BF16, name=f"xr{b}{h}")
                       for h in range(2)] for b in range(NIMG)]
        trashA = sb.tile([128, 1024], BF16)
        trashD = sb.tile([128, 1024], BF16)
        trashP = sb.tile([128, 1024], BF16)
        trashf = sb.tile([128, 2304], F32)

        # ---- input DMAs, spread over 4 queues so startup transfers overlap ----
        # sync: img0 by w-chunk (P1 consumes per-wb), then img1/sd
        img0 = sbi.tile([128, 3, 3, 3, 128], BF16, tag="img", bufs=2)
        for wb in range(3):
            nc.sync.dma_start(out=img0[:, wb], in_=imgs_d[0][:, wb])
        # act queue: first conv band (needed by P1 immediately)
        nc.scalar.dma_start(out=b1sp_t, in_=b1sp_d[:, :, :])
        nc.scalar.dma_start(out=mgap_t, in_=mgap_d[:, :, :])
        # pool queue: second band, P2 matrices, descriptors, gauss bands
        img_tiles, sd_tiles, dsl_tiles = [], [], []
        nc.gpsimd.dma_start(out=b1dp_t, in_=b1dp_d[:, :, :])
        nc.gpsimd.dma_start(out=msm_t, in_=msm_d[:, :, :])
        nc.gpsimd.dma_start(out=mdf_t, in_=mdf_d[:, :, :])
        for b in range(NIMG):
            dsl = sbi.tile([128, 2, NDESC], FP8, tag="dsl", bufs=2)
            nc.gpsimd.dma_start(out=dsl, in_=dsl_d[b])
            dsl_tiles.append(dsl)
        nc.gpsimd.dma_start(out=mga_t, in_=mga_d[:, :, :])
        nc.gpsimd.dma_start(out=mgan_t, in_=mgan_d[:, :, :])
        nc.gpsimd.dma_start(out=mga2_t, in_=mga2_d[:, :, :])
        sdall = sbi.tile([128, 6, 384], BF16, tag="sdall", bufs=1)
        for b in range(NIMG):
            nc.sync.dma_start(out=sdall[:, 3 * b:3 * b + 3, :],
                              in_=sd_d[b])
            sd_tiles.append(sdall[:, 3 * b:3 * b + 3, :])
        img1 = sbi.tile([128, 3, 3, 3, 128], BF16, tag="img", bufs=2)
        nc.sync.dma_start(out=img1, in_=imgs_d[1])
        img_tiles.extend([img0, img1])
        for t, d in [(s8b_t, s8b_d), (ident_t, ident_d),
                     (t16_t, t16_d), (ninfh_t, ninfh_d),
                     (thr128_t, thr128_d), (ones128_t, ones128_d)]:
            nc.sync.dma_start(out=t, in_=d[:, :])

        # ---- gram tile generator (upper triangle, packed 1024-wide) ----
        def gram_tiles_gen():
            for b in range(NIMG):
                for kind, segs in GRAM_TILES:
                    yield (b, kind, segs)

        _gram_iter = gram_tiles_gen()
        _gram_state = {"done": False, "i": 0, "nd": 0, "ns": 0}

        def pump_gram(n, pat=None):
            for _ in range(n):
                if _gram_state["done"]:
                    break
                try:
                    b, kind, segs = next(_gram_iter)
                except StopIteration:
                    _gram_state["done"] = True
                    break
                dsl = dsl_tiles[b]
                gp = ps_g.tile([128, 1024], F32, tag="g")
                off = 0
                for (bi, pos, ln) in segs:
                    o = 0
                    while o < ln:
                        nn = min(512, ln - o)
                        nc.tensor.matmul(gp[:, off:off + nn],
                                         dsl[:, :, 128 * bi:128 * (bi + 1)],
                                         dsl[:, :, pos + o:pos + o + nn],
                                         start=True, stop=True, perf_mode=DR)
                        o += nn
                        off += nn
                col = _gram_state["i"]
                eng = ENG_PATTERN[col % len(ENG_PATTERN)]
                if kind == "diag":
                    acc = gdacc[:, _gram_state["nd"]:_gram_state["nd"] + 1]
                    _gram_state["nd"] += 1
                else:
                    acc = gracc[:, _gram_state["ns"]:_gram_state["ns"] + 1]
                    _gram_state["ns"] += 1
                if eng == 'A':
                    nc.scalar.activation(trashA, gp, AF.Relu, accum_out=acc)
                else:
                    nc.vector.tensor_scalar(trashD, gp, 0.0, None,
                                            op0=ALU.max, op1=ALU.add,
                                            accum_out=acc)
                _gram_state["i"] += 1

        # ---- conv + NMS per image ----
        resp_list, nms_list, p16s_list, csd_list = [], [], [], []

        def emit_conv(b, pump=True, fast=False):
            img_t = img_tiles[b]
            imgv = img_t.rearrange("p wb c hc w -> p wb (c hc) w")

            # P1: smooth/diff along H -> [w-part, wb, h]
            sT = sbi.tile([128, 3, 384], BF16, tag="sT")
            dT = sbi.tile([128, 3, 384], BF16, tag="dT")
            for wb in range(3):
                for di, (dst, bnd, wins) in enumerate(
                        ((sT, b1sp_t, w1s), (dT, b1dp_t, w1d))):
                    pst = ps_cv.tile([128, 384], F32, tag="cv")
                    first = True
                    for k in range(9):
                        if wins[k] is None:
                            continue
                        c0, c1 = wins[k]
                        nc.tensor.matmul(pst[:, c0:c1],
                                         imgv[:, wb, k, :],
                                         bnd[:, k, 0:c1 - c0], start=first, stop=False)
                        first = False
                    if (di * 3 + wb) % 2 == 0:
                        nc.scalar.copy(dst[:, wb, :], pst)
                    else:
                        nc.vector.tensor_copy(dst[:, wb, :], pst)

            # P2: diff/smooth along W -> px=dx, py=dy [w-part(ob), h]
            pq = sbi.tile([128, 3, 384], BF16, tag="pq", bufs=2)
            qq = sbi.tile([128, 3, 384], BF16, tag="qq", bufs=2)
            rr = sbi.tile([128, 3, 384], BF16, tag="rr", bufs=2)
            for ob in range(3):
                px = ps_cv.tile([128, 384], F32, tag="cv")
                fx = True
                for kc, ob2 in pr_df:
                    if ob2 != ob:
                        continue
                    nc.tensor.matmul(px, mdf_t[:, kc, ob * 128:(ob + 1) * 128],
                                     sT[:, kc, :], start=fx, stop=False)
                    fx = False
                nc.scalar.activation(pq[:, ob, :], px, AF.Square)
                py = ps_cv.tile([128, 384], F32, tag="cv")
                fy = True
                for kc, ob2 in pr_sm:
                    if ob2 != ob:
                        continue
                    nc.tensor.matmul(py, msm_t[:, kc, ob * 128:(ob + 1) * 128],
                                     dT[:, kc, :], start=fy, stop=False)
                    fy = False
                nc.scalar.activation(qq[:, ob, :], py, AF.Square)
                # (dx+dy)^2 channel: continue accumulating dy into px's psum
                # (after pq extracted it) instead of re-multiplying the dx part
                for kc, ob2 in pr_sm:
                    if ob2 != ob:
                        continue
                    nc.tensor.matmul(px, msm_t[:, kc, ob * 128:(ob + 1) * 128],
                                     dT[:, kc, :], start=False, stop=False)
                nc.scalar.activation(rr[:, ob, :], px, AF.Square)
            if pump:
                pump_gram(2)

            # P3: gauss along W -> [h-part(hb), w]
            gP = sbi.tile([128, 3, 384], BF16, tag="gP", bufs=2)
            gM = sbi.tile([128, 3, 384], BF16, tag="gM", bufs=2)
            gR = sbi.tile([128, 3, 384], BF16, tag="gR", bufs=2)
            for si, (src, dst) in enumerate(((pq, gP), (qq, gM), (rr, gR))):
                for hb in range(3):
                    pst = ps_cv.tile([128, 384], F32, tag="cv")
                    for i, cw in enumerate(range(3)):
                        c0, c1 = wga3[cw]
                        nc.tensor.matmul(pst[:, c0:c1],
                                         src[:, cw, hb * 128:(hb + 1) * 128],
                                         mgap_t[:, cw, 0:c1 - c0],
                                         start=(i == 0), stop=False)
                    if (si * 3 + hb) % 3 != 2:
                        nc.vector.tensor_copy(dst[:, hb, :], pst)
                    else:
                        nc.scalar.copy(dst[:, hb, :], pst)

            if pump:
                pump_gram(2)

            # P4: gauss along H -> P (tr), M, R in [h-part(ob), w]; response
            resp = sbi.tile([128, 3, 388], BF16, tag="resp", bufs=2)
            for ob in range(3):
                pR = ps_cv.tile([128, 384], F32, tag="cv")
                first = True
                for kc, ob2 in pr_ga:
                    if ob2 != ob:
                        continue
                    nc.tensor.matmul(pR, mga_t[:, kc, ob * 128:(ob + 1) * 128],
                                     gR[:, kc, :], start=first, stop=False)
                    first = False
                for kc, ob2 in pr_ga:
                    if ob2 != ob:
                        continue
                    nc.tensor.matmul(pR, mgan_t[:, kc, ob * 128:(ob + 1) * 128],
                                     gP[:, kc, :], start=False, stop=False)
                    nc.tensor.matmul(pR, mgan_t[:, kc, ob * 128:(ob + 1) * 128],
                                     gM[:, kc, :], start=False, stop=False)
                B4 = sbi.tile([128, 384], BF16, tag="B4", bufs=2)
                nc.scalar.activation(B4, pR, AF.Square)
                pM = ps_cv.tile([128, 384], F32, tag="cv")
                first = True
                for kc, ob2 in pr_ga:
                    if ob2 != ob:
                        continue
                    nc.tensor.matmul(pM, mga_t[:, kc, ob * 128:(ob + 1) * 128],
                                     gP[:, kc, :], start=first, stop=False)
                    first = False
                for kc, ob2 in pr_ga:
                    if ob2 != ob:
                        continue
                    nc.tensor.matmul(pM, mgan_t[:, kc, ob * 128:(ob + 1) * 128],
                                     gM[:, kc, :], start=False, stop=False)
                A2 = sbi.tile([128, 384], BF16, tag="A2", bufs=2)
                nc.scalar.activation(A2, pM, AF.Square)
                disc = sbi.tile([128, 384], BF16, tag="disc", bufs=2)
                deng = nc.vector if fast else nc.gpsimd
                deng.tensor_tensor(out=disc, in0=A2, in1=B4, op=ALU.add)
                s2 = sbi.tile([128, 384], F32, tag="s2", bufs=2)
                nc.scalar.activation(s2, disc, AF.Sqrt, scale=0.25)
                for kc, ob2 in pr_ga:
                    if ob2 != ob:
                        continue
                    nc.tensor.matmul(pM, mga2_t[:, kc, ob * 128:(ob + 1) * 128],
                                     gM[:, kc, :], start=False, stop=False)
                nc.vector.scalar_tensor_tensor(out=resp[:, ob, 2:386], in0=pM,
                                               scalar=0.5, in1=s2,
                                               op0=ALU.mult, op1=ALU.subtract)
                nc.vector.tensor_copy(resp[:, ob, 0:2], ninfh_t[:, 0:2])
                nc.vector.tensor_copy(resp[:, ob, 386:388], ninfh_t[:, 0:2])
            resp_list.append(resp)

        def emit_softplus():
            sdv = sdall.rearrange("p c w -> p (c w)")
            spA = sbi.tile([128, 2304], F32, tag="spA", bufs=1)
            nc.scalar.activation(spA, sdv, AF.Exp)
            nc.scalar.activation(trashf, spA, AF.Ln, bias=1.0,
                                 accum_out=spacc[:, 0:1])

        def emit_nms_gen(b, fast=False):
            TTs = nc.vector.tensor_tensor
            resp = resp_list[b]
            t1 = sbi.tile([128, 3, 388], BF16, tag="t1", bufs=2)
            TTs(out=t1[:, :, 0:387], in0=resp[:, :, 0:387],
                in1=resp[:, :, 1:388], op=ALU.max)
            t2 = sbi.tile([128, 3, 388], BF16, tag="t2", bufs=2)
            TTs(out=t2[:, :, 0:385], in0=t1[:, :, 0:385],
                in1=t1[:, :, 2:387], op=ALU.max)
            m1 = sbi.tile([128, 3, 384], BF16, tag="m1", bufs=2)
            TTs(out=m1, in0=t2[:, :, 0:384], in1=resp[:, :, 4:388], op=ALU.max)
            yield
            shs = []
            for k in (1, 2):
                q1dn = nc.sync if k == 1 else nc.gpsimd
                sh = sbi.tile([128, 3, 384], BF16, tag="shd", bufs=2)
                q1dn.dma_start(out=sh[0:k, 0, :], in_=ninfh_t[0:k, 0:384])
                q1dn.dma_start(out=sh[k:128], in_=m1[0:128 - k])
                q1dn.dma_start(out=sh[0:k, 1:3, :], in_=m1[128 - k:128, 0:2, :])
                shs.append(sh)
                q1up = nc.gpsimd if k == 1 else nc.sync
                sh2 = sbi.tile([128, 3, 384], BF16, tag="shu", bufs=2)
                q1up.dma_start(out=sh2[128 - k:128, 2, :],
                               in_=ninfh_t[0:k, 0:384])
                q1up.dma_start(out=sh2[0:128 - k], in_=m1[k:128])
                q1up.dma_start(out=sh2[128 - k:128, 0:2, :], in_=m1[0:k, 1:3, :])
                shs.append(sh2)
            yield
            mp1 = sbi.tile([128, 3, 384], BF16, tag="mp1", bufs=2)
            nc.vector.tensor_tensor(out=mp1, in0=m1, in1=shs[0], op=ALU.max)
            mp2 = sbi.tile([128, 3, 384], BF16, tag="mp2", bufs=2)
            TTs(out=mp2, in0=shs[1], in1=shs[2], op=ALU.max)
            nc.vector.tensor_tensor(out=mp2, in0=mp2, in1=shs[3], op=ALU.max)
            mp = sbi.tile([128, 3, 384], BF16, tag="mp", bufs=2)
            nc.vector.tensor_tensor(out=mp, in0=mp1, in1=mp2, op=ALU.max)
            yield
            e1 = sbi.tile([128, 3, 384], BF16, tag="e1", bufs=2)
            TTs(out=e1, in0=resp[:, :, 2:386], in1=mp, op=ALU.is_ge)
            nms = sbi.tile([128, 3, 384], BF16, tag="nms", bufs=2)
            neng = nc.vector if fast else nc.gpsimd
            neng.tensor_tensor(out=nms, in0=resp[:, :, 2:386], in1=e1, op=ALU.mult)
            nms_list.append(nms)
            yield
            # 8-wide block max via 3 strided 2x TT ops
            nv = nms.rearrange("p c (g k) -> p c g k", k=8)
            b4m = sbi.tile([128, 3, 48, 4], BF16, tag="b4m", bufs=2)
            TTs(out=b4m, in0=nv[:, :, :, 0:4], in1=nv[:, :, :, 4:8], op=ALU.max)
            b2m = sbi.tile([128, 3, 48, 2], BF16, tag="b2m", bufs=2)
            TTs(out=b2m, in0=b4m[:, :, :, 0:2], in1=b4m[:, :, :, 2:4], op=ALU.max)
            bw = sbi.tile([128, 3, 48], BF16, tag="bw")
            TTs(out=bw.rearrange("p c g -> p c g ()"),
                in0=b2m[:, :, :, 0:1], in1=b2m[:, :, :, 1:2], op=ALU.max)
            # H-direction 8-max: transpose so H is the free axis, tree-max,
            # transpose back to hb-major
            bwf = bw.rearrange("p c g -> p (c g)")
            pT0 = ps_cv.tile([128, 128], BF16, tag="cv")
            nc.tensor.transpose(pT0, bwf[:, 0:128], ident_t)
            pT1 = ps_cv.tile([16, 128], BF16, tag="cv")
            nc.tensor.transpose(pT1, bwf[:, 128:144], ident_t)
            yield
            bT0 = sbi.tile([128, 16, 8], BF16, tag="bT0", bufs=2)
            nc.vector.tensor_copy(bT0.rearrange("p a k -> p (a k)"), pT0)
            bT1 = sbi.tile([16, 16, 8], BF16, tag="bT1", bufs=2)
            nc.scalar.copy(bT1.rearrange("p a k -> p (a k)"), pT1)
            m4a = sbi.tile([128, 16, 4], BF16, tag="m4a", bufs=2)
            TTs(out=m4a, in0=bT0[:, :, 0:4], in1=bT0[:, :, 4:8], op=ALU.max)
            m2a = sbi.tile([128, 16, 2], BF16, tag="m2a", bufs=2)
            TTs(out=m2a, in0=m4a[:, :, 0:2], in1=m4a[:, :, 2:4], op=ALU.max)
            p16T0 = sbi.tile([128, 16], BF16, tag="p16T0", bufs=2)
            TTs(out=p16T0.rearrange("p a -> p a ()"),
                in0=m2a[:, :, 0:1], in1=m2a[:, :, 1:2], op=ALU.max)
            m4b = sbi.tile([16, 16, 4], BF16, tag="m4b", bufs=2)
            TTs(out=m4b, in0=bT1[:, :, 0:4], in1=bT1[:, :, 4:8], op=ALU.max)
            m2b = sbi.tile([16, 16, 2], BF16, tag="m2b", bufs=2)
            TTs(out=m2b, in0=m4b[:, :, 0:2], in1=m4b[:, :, 2:4], op=ALU.max)
            p16T1 = sbi.tile([16, 16], BF16, tag="p16T1", bufs=2)
            TTs(out=p16T1.rearrange("p a -> p a ()"),
                in0=m2b[:, :, 0:1], in1=m2b[:, :, 1:2], op=ALU.max)
            yield
            q0 = ps_cv.tile([16, 128], BF16, tag="cv")
            nc.tensor.transpose(q0, p16T0, ident_t)
            q1 = ps_cv.tile([16, 16], BF16, tag="cv")
            nc.tensor.transpose(q1, p16T1, ident_t[0:16, 0:16])
            p16s = sbi.tile([16, 3, 48], BF16, tag="p16s", bufs=2)
            p16f = p16s.rearrange("p c g -> p (c g)")
            nc.vector.tensor_copy(p16f[:, 0:128], q0)
            nc.scalar.copy(p16f[:, 128:144], q1)
            xra, xrb = xrow_tiles[b]
            nc.sync.dma_start(out=xra[0:1, 0:576], in_=p16f[0:4, :])
            nc.scalar.dma_start(out=xra[0:1, 576:1152], in_=p16f[4:8, :])
            nc.gpsimd.dma_start(out=xrb[0:1, 0:576], in_=p16f[8:12, :])
            nc.sync.dma_start(out=xrb[0:1, 576:1152], in_=p16f[12:16, :])
            p16s_list.append(p16s)

        # ---- per-image threshold search + selection (interleavable) ----
        WT = 0.25 / 128.0
        maskz = sb.tile([128, 2], F32)   # per-image threshold masks
        trash128 = sb.tile([128, 2304], BF16)
        nc.vector.memset(vals[:, 1:3], 0.0)

        def count_gen(b):
            xra, xrb = xrow_tiles[b]
            x128 = sbi.tile([128, 2304], BF16, tag="x128", bufs=2)
            nc.gpsimd.partition_broadcast(x128[:, 0:1152], xra, channels=128)
            yield
            cnta = sbi.tile([128, 2], F32, tag="cnta", bufs=2)
            nc.vector.tensor_scalar(trash128[:, 0:1152], x128[:, 0:1152],
                                    thr128_t[:, 0:1], None,
                                    op0=ALU.is_gt, op1=ALU.add,
                                    accum_out=cnta[:, 0:1])
            nc.gpsimd.partition_broadcast(x128[:, 1152:2304], xrb, channels=128)
            yield
            nc.vector.tensor_scalar(trash128[:, 1152:2304], x128[:, 1152:2304],
                                    thr128_t[:, 0:1], None,
                                    op0=ALU.is_gt, op1=ALU.add,
                                    accum_out=cnta[:, 1:2])
            cnt = sbi.tile([128, 1], F32, tag="cnt", bufs=2)
            nc.vector.tensor_tensor(out=cnt, in0=cnta[:, 0:1], in1=cnta[:, 1:2],
                                    op=ALU.add)
            nc.vector.tensor_scalar(maskz[:, b:b + 1], cnt, float(NUM) - 0.5,
                                    None, op0=ALU.is_ge)
            yield
            kpa = sbi.tile([128, 1], F32, tag="kpa", bufs=2)
            nc.gpsimd.partition_all_reduce(kpa, maskz[:, b:b + 1], channels=128,
                                           reduce_op=bass_isa.ReduceOp.add)
            tb16 = sbi.tile([16, 1], F32, tag="tb16", bufs=2)
            nc.vector.tensor_scalar(tb16, kpa[0:16], WT, -WT,
                                    op0=ALU.mult, op1=ALU.add)
            nc.vector.tensor_scalar(tb16, tb16, 1e-30, None, op0=ALU.max)
            tb_list[b] = tb16

        def emit_csd(b, fast=False):
            # pixel==block-max mask (threshold-free) and its sd product
            nms = nms_list[b]
            sdt = sd_tiles[b]
            p16s = p16s_list[b]
            c1 = sbi.tile([128, 3, 384], BF16, tag="c1", bufs=2)
            for cc in range(3):
                bexp0 = ps_cv.tile([128, 384], F32, tag="cv")
                nc.tensor.matmul(bexp0, t16_t,
                                 p16s[:, cc, :].unsqueeze(2)
                                 .to_broadcast([16, 48, 8]),
                                 start=True, stop=True)
                nc.vector.tensor_tensor(out=c1[:, cc, :], in0=nms[:, cc, :],
                                        in1=bexp0, op=ALU.is_ge)
            eng = nc.vector if fast else nc.gpsimd
            csd = sbi.tile([128, 3, 384], BF16, tag="csd", bufs=2)
            eng.tensor_tensor(out=csd, in0=c1, in1=sdt, op=ALU.mult)
            # per-block sums of csd: W-tree then partition-group matvec
            cv8 = csd.rearrange("p c (g k) -> p c g k", k=8)
            s4 = sbi.tile([128, 3, 48, 4], BF16, tag="s4", bufs=2)
            eng.tensor_tensor(out=s4, in0=cv8[:, :, :, 0:4],
                              in1=cv8[:, :, :, 4:8], op=ALU.add)
            s2 = sbi.tile([128, 3, 48, 2], BF16, tag="s2b", bufs=2)
            eng.tensor_tensor(out=s2, in0=s4[:, :, :, 0:2],
                              in1=s4[:, :, :, 2:4], op=ALU.add)
            csdW = sbi.tile([128, 3, 48], BF16, tag="csdW", bufs=2)
            eng.tensor_tensor(out=csdW.rearrange("p c g -> p c g ()"),
                              in0=s2[:, :, :, 0:1], in1=s2[:, :, :, 1:2],
                              op=ALU.add)
            bsdP = ps_m.tile([16, 144], F32, tag="m", bufs=1)
            nc.tensor.matmul(bsdP, s8b_t, csdW.rearrange("p c g -> p (c g)"),
                             start=True, stop=True)
            csd_list.append(bsdP)

        def emit_dsel(b):
            # fused block-level selection + dot: ((blockmax >= tb) * blocksum)
            dsel = sbi.tile([16, 144], BF16, tag="dsel", bufs=2)
            nc.vector.scalar_tensor_tensor(
                out=dsel, in0=p16s_list[b].rearrange("p c g -> p (c g)"),
                scalar=tb_list[b][:, 0:1],
                in1=csd_list[b], op0=ALU.is_ge, op1=ALU.mult,
                accum_out=vals[0:16, 1 + b:2 + b])

        tb_list = [None, None]

        # ================= schedule =================
        def drive(gen, tiles_per_step=1, pat=None):
            for _ in gen:
                pump_gram(tiles_per_step, pat=pat)

        def interleave(ga, gb, tiles_per_step=1, pat=None):
            act = [ga, gb]
            while act:
                for g in list(act):
                    try:
                        next(g)
                    except StopIteration:
                        act.remove(g)
                        continue
                    pump_gram(tiles_per_step, pat=pat)

        emit_conv(0, pump=False)
        pump_gram(2)
        emit_conv(1, fast=False)
        emit_softplus()
        drive(emit_nms_gen(0), 1)
        interleave(count_gen(0), emit_nms_gen(1, fast=False), 1)
        emit_csd(0)
        emit_dsel(0)
        drive(count_gen(1), 0)
        pump_gram(1000)
        emit_csd(1, fast=True)
        emit_dsel(1)

        # ---- final reduce ----
        nc.vector.tensor_copy(vals[:, 0:1], spacc)
        nc.vector.tensor_reduce(vals[:, 3:4], gracc, axis=AX.X, op=ALU.add)
        nc.vector.tensor_reduce(vals[:, 4:5], gdacc, axis=AX.X, op=ALU.add)
        fps = ps_m.tile([5, 1], F32, tag="m", bufs=1)
        nc.tensor.matmul(fps, vals, ones128_t, start=True, stop=True)
        fsb = sb.tile([5, 1], F32)
        nc.vector.tensor_copy(fsb, fps)
        nc.sync.dma_start(out=out_d[:, :], in_=fsb)

        ps_m.release()
        ps_g.release()
        ps_cv.release()
        sbi.release()
        sb.release()

    nc.finalize()
    return nc, C


_CACHE = {}


def kernel(descriptors, scores, scores_dense, imgs):
    B = descriptors.shape[0]
    ncore = 8
    per = B // ncore
    if "nc" not in _CACHE:
        _CACHE["nc"], _CACHE["C"] = build_program()
    nc, C = _CACHE["nc"], _CACHE["C"]

    imgs_r = (np.asarray(imgs).reshape(B, 3, 3, 128, 3, 128)
              .transpose(0, 3, 4, 1, 2, 5))
    imgs_bf = np.ascontiguousarray(imgs_r.astype(ml_dtypes.bfloat16))
    sd_r = (np.asarray(scores_dense).reshape(B, 3, 128, W)
            .transpose(0, 2, 1, 3))
    sd = np.ascontiguousarray(sd_r.astype(ml_dtypes.bfloat16))
    desc = np.asarray(descriptors, dtype=np.float32)
    nrm = np.sqrt(np.einsum('bnd,bnd->bn', desc, desc))
    nrm = np.maximum(nrm, 1e-8)
    desc8 = (desc / nrm[:, :, None]).astype(ml_dtypes.float8_e4m3)
    # slab d-major: [B, 128(dj), 2(slab), N]
    dsl = np.ascontiguousarray(desc8.transpose(0, 2, 1)
                               .reshape(B, 2, 128, NDESC).transpose(0, 2, 1, 3))

    in_maps = []
    for c in range(ncore):
        s = slice(c * per, (c + 1) * per)
        in_maps.append({
            "imgs": imgs_bf[s], "sd": sd[s], "dsl": dsl[s],
            "b1sp": C["b1sp"], "b1dp": C["b1dp"], "mgap": C["mgap"],
            "msm": C["msm"], "mdf": C["mdf"], "mga": C["mga"], "mgan": C["mgan"],
            "mga2": C["mga2"],
            "s8b": C["s8b"], "ident": C["ident"],
            "t16": C["t16"], "ninfh": C["ninfh"],
            "thr128": C["thr128"], "ones128": C["ones128"],
        })

    res = run_bass_kernel_spmd(nc, in_maps, core_ids=list(range(ncore)))
    S1 = S2 = Sg = Sd = 0.0
    for c in range(ncore):
        o = np.asarray(res.results[c]["out"])[:, 0].astype(np.float64)
        S1 += o[0]
        S2 += o[1] + o[2]
        Sg += o[3]
        Sd += o[4]
    bce = (S1 - S2) / (B * H * W)
    relu_mean = (2.0 * Sg + Sd) / (B * NDESC * NDESC)
    return np.array(bce + relu_mean, dtype=np.float32)
